# revision 22
# baseline (speedup 1.0000x reference)
"""Trainium2 Bass kernel for the box-ranking depth loss.

Math restructuring (vs the reference):
  - Global min-max normalization is affine; per-box stats of normalized
    depth are recovered from raw-depth stats (the affine constants cancel
    in the loss terms), so each core only needs raw per-box
    {sum, sumsq, min, max} plus the global {min, max}.
  - Box sums/sumsq (exact): per-row f32 prefix sums -> per-box prefix
    differences at the static column edges; the per-box prefix columns
    are fetched with gpsimd ap_gather (one instruction for all 32 boxes)
    -> one subtract + row mask -> cross-row reduction with gpsimd
    partition_all_reduce (no PE transpose round-trip).
  - Box min/max (approximate): column ranges expanded to 16-col block
    boundaries (<= 15 extra cols per side; only perturbs the bmax-bmin
    denominator, ~1e-3 rel on loss_std vs the 2e-2 gate -- dominated by
    the fp16 table rounding, not the expansion).  Block min/max built as
    a j-interleaved packed-pair fp16 pyramid (every level runs at DVE 2x)
    + one stride-2 merge; the max side is negated once at block level so
    block-domain sliding-window doubling and the per-box lookups use MIN
    ops for both sides; ONE strided reduce per box (boxes with equal
    block width share one reduce via an extra AP dim) yields
    (mincand, -maxcand).  All later combines are MAX of negated values.
  - Cross-partition and cross-core combines use partition_all_reduce /
    partition_broadcast; final scalar math is in row form on partition 0.

Sharding: rows (H) split 8 ways -> each core holds a [128, 2048] slab.
Two AllGathers (sums early, min/max late).  Every core redundantly
computes the final 3-float loss vector.
"""

import numpy as np

H, W, T, NCORES = 1024, 2048, 32, 8
R = H // NCORES          # 128 rows per core
BIG = 1e30
RATIO = 1.0
NB = W // 16             # 128 column blocks of 16
KB = 8                   # lookup window = 8 blocks = 128 cols
NMM = 2 * T + 2          # 66 min/max stat columns
NSTAT = 2 * T + NMM      # 130 total stat columns
DIN_W = W + NMM + 2 * T  # slab | rneg(66) | rinddup(64)
CST_W = 264
PSOFF = 2                # zero cols at the head of ps12 (x1==0 gathers)


def _win_view(tab_ap, b1, b2, k, ap_ctor, pair_stride):
    """AP over block-domain sliding-window tables: windows of k blocks
    covering [b1, b2) (two step-k phases when k does not divide), with an
    outer [pair_stride, 2] dim pairing the min table with the negated max
    table so one MIN reduce serves both sides."""
    q = (b2 - b1) - k
    n = q // k + 1
    s1 = q - k * (n - 1)
    base = tab_ap[:, 0:1]
    ppair = list(base.ap[0])
    dims = [ppair, [pair_stride, 2]]
    if s1 != 0:
        dims.append([s1, 2])
    dims.append([k, n])
    return ap_ctor(base.tensor, base.offset + b1, dims), (s1 != 0)


def _build_program(bboxes, single_core=False, reps=1, mock_cc=False):
    import concourse.bacc as bacc
    import concourse.mybir as mybir
    import concourse.tile as tile
    from concourse.ap import AP
    from concourse.alu_op_type import AluOpType as alu
    from concourse import bass_isa

    f32 = mybir.dt.float32
    f16 = mybir.dt.float16
    X = mybir.AxisListType.X
    XY = mybir.AxisListType.XY
    AF = mybir.ActivationFunctionType
    RO = bass_isa.ReduceOp

    x1s, x2s = bboxes[:, 0], bboxes[:, 2]
    xa1 = (x1s // 16).astype(int)           # block-aligned box edges
    xa2 = ((x2s + 15) // 16).astype(int)
    GVIEW = int(max(x2s)) + PSOFF           # gather view covers all idxs

    nc = bacc.Bacc("TRN2", target_bir_lowering=False, debug=False,
                   num_devices=1 if single_core else NCORES)

    din = nc.dram_tensor("din", [R, DIN_W], f32, kind="ExternalInput").ap()
    cst = nc.dram_tensor("cst", [128, CST_W], f32, kind="ExternalInput").ap()
    out = nc.dram_tensor("out", [3], f32, kind="ExternalOutput").ap()

    def sb(name, shape, dt=f32):
        return nc.alloc_sbuf_tensor(name, shape, dt).ap()

    ds = sb("ds", [R, DIN_W])            # slab + masks
    cstS = sb("cstS", [128, CST_W])
    ds2 = sb("ds2", [R, W])
    ps12 = sb("ps12", [R, 2 * PSOFF + 2 * W])  # [0 0 |ps| 0 0 |ps2]
    dsH = sb("dsH", [R, W], f16)
    p1n = sb("p1n", [R, W // 2], f16)
    p1x = sb("p1x", [R, W // 2], f16)
    p2n = sb("p2n", [R, W // 4], f16)
    p2x = sb("p2x", [R, W // 4], f16)
    p3n = sb("p3n", [R, W // 8], f16)
    p3x = sb("p3x", [R, W // 8], f16)
    b16n = sb("b16n", [R, NB], f16)
    b16x = sb("b16x", [R, NB], f16)
    nbx = sb("nbx", [R, NB], f16)        # negated max blocks
    dn2 = sb("dn2", [R, NB], f16)
    dn4 = sb("dn4", [R, NB], f16)
    dx2 = sb("dx2", [R, NB], f16)
    dx4 = sb("dx4", [R, NB], f16)
    tab = sb("tab", [R, 2 * NB], f16)    # [D8n | D8x] adjacent
    gx12 = sb("gx12", [R, 4 * T])
    rs12 = sb("rs12", [R, 2 * T])
    rrs12 = sb("rrs12", [R, 2 * T])
    rmm = sb("rmm", [R, NMM])            # [mincand(32)|gn| -maxcand(32)|gx]
    stkv = sb("stkv", [R, NMM])
    statT = sb("statT", [128, NSTAT])    # PAR outputs: [sums|sumsq|minmax]
    gath = sb("gath", [NCORES, NSTAT])
    redT = sb("redT", [NCORES, NSTAT])
    meanR = sb("meanR", [1, T])
    a2R = sb("a2R", [1, T])
    mBR = sb("mBR", [1, T])
    varR = sb("varR", [1, T])
    stdR = sb("stdR", [1, T])
    rngR = sb("rngR", [1, T + 1])
    rinvR = sb("rinvR", [1, T + 1])
    srvR = sb("srvR", [1, T])
    qm = sb("qm", [T, T])
    t2m = sb("t2m", [T, T])
    t3m = sb("t3m", [T, T])
    raccv = sb("raccv", [T, 1])
    dummy = sb("dmy0", [1, 8])
    out3 = sb("out3", [1, 4])

    # const views
    gmatC = cstS[0:T, 128:160]
    cntinvR = cstS[0:1, 160:160 + T]
    cm1invR = cstS[0:1, 192:192 + T]
    onesRow = cstS[0:1, 224:224 + T]
    oneOne = cstS[0:1, 224:225]
    ones32c = cstS[0:T, 226:227]
    idxC = cstS[:, 256:258]

    rnegS = ds[:, W:W + NMM]
    rindD = ds[:, W + NMM:W + NMM + 2 * T]

    Q = W // 4
    CH = [0, 512, 1024, 1536, 2048]  # DMA/compute chunk bounds

    def pairs4(src, off, cnt):
        # packed-pair view {4b+off, 4b+1+off : b < cnt} -> [R, cnt, 2], 2x
        base = src[:, 0:1]
        pp = list(base.ap[0])
        return AP(base.tensor, base.offset + off, [pp, [4, cnt], [1, 2]])

    def stride2(src, off, cnt):
        base = src[:, 0:1]
        pp = list(base.ap[0])
        return AP(base.tensor, base.offset + off, [pp, [2, cnt]])

    with tile.TileContext(nc) as tc:
        with tc.tile_pool(name="psum", bufs=1, space="PSUM") as pp, \
                tc.tile_pool(name="dram", bufs=1, space="DRAM") as dram:
            mrB = pp.tile([T, T], f32, name="mrB")
            mcolP = pp.tile([T, 1], f32, name="mcolP")
            aCol = pp.tile([T, 1], f32, name="aCol")
            accP = pp.tile([1, 1], f32, name="accP")

            cstatS = dram.tile([1, 2 * T], f32, name="cstatS")
            cgathS = dram.tile([NCORES, 2 * T], f32, name="cgathS")
            cstatM = dram.tile([1, NMM], f32, name="cstatM")
            cgathM = dram.tile([NCORES, NMM], f32, name="cgathM")

            for _rep in range(reps):
                # ---- ACT function-table preloads (overlap the input DMA) ----
                nc.vector.memset(dummy[0:1, 0:1], 0.0)
                nc.scalar.activation(dummy[0:1, 1:2], dummy[0:1, 0:1], AF.Square)
                nc.scalar.activation(dummy[0:1, 2:3], dummy[0:1, 0:1], AF.Sqrt)
                nc.scalar.activation(dummy[0:1, 3:4], dummy[0:1, 0:1], AF.Relu)

                # ---- loads (quarters, alternating the two HWDGE queues) ----
                for qi in range(4):
                    eng = nc.sync if qi % 2 == 0 else nc.scalar
                    eng.dma_start(out=ds[:, CH[qi]:CH[qi + 1]],
                                  in_=din[:, CH[qi]:CH[qi + 1]])
                nc.sync.dma_start(out=ds[:, W:DIN_W], in_=din[:, W:DIN_W])
                nc.scalar.dma_start(out=cstS[:], in_=cst[:])

                # ---- ACT: per-quarter squares and fp16 copies ----
                for qi in range(4):
                    a, b = CH[qi], CH[qi + 1]
                    nc.scalar.square(ds2[:, a:b], ds[:, a:b])
                    nc.scalar.copy(dsH[:, a:b], ds[:, a:b])

                # ---- DVE: row prefix sums (f32 scans) ----
                nc.gpsimd.memset(ps12[:, 0:PSOFF], 0.0)
                nc.gpsimd.memset(ps12[:, PSOFF + W:2 * PSOFF + W], 0.0)
                for qi in range(4):
                    a, b = PSOFF + CH[qi], PSOFF + CH[qi + 1]
                    nc.vector.tensor_tensor_scan(
                        ps12[:, a:b], ds[:, CH[qi]:CH[qi + 1]],
                        ds[:, CH[qi]:CH[qi + 1]],
                        0.0 if qi == 0 else ps12[:, a - 1:a],
                        alu.add, alu.bypass)
                # Pool: gather the sum prefix cols per box (hi x 32 | lo x 32)
                nc.gpsimd.ap_gather(gx12[:, 0:2 * T],
                                    ps12[:, 0:GVIEW],
                                    idxC.bitcast(mybir.dt.int16),
                                    128, GVIEW, 1, 2 * T)
                off2 = 2 * PSOFF + W
                for qi in range(4):
                    a, b = off2 + CH[qi], off2 + CH[qi + 1]
                    nc.vector.tensor_tensor_scan(
                        ps12[:, a:b], ds2[:, CH[qi]:CH[qi + 1]],
                        ds2[:, CH[qi]:CH[qi + 1]],
                        0.0 if qi == 0 else ps12[:, a - 1:a],
                        alu.add, alu.bypass)
                nc.gpsimd.ap_gather(gx12[:, 2 * T:4 * T],
                                    ps12[:, PSOFF + W:PSOFF + W + GVIEW],
                                    idxC.bitcast(mybir.dt.int16),
                                    128, GVIEW, 1, 2 * T)

                # ---- DVE: block min/max pyramid + sliding windows ----
                with nc.allow_low_precision(reason="fp16 min/max tables"):
                    # j-interleaved packed-pair pyramid (fp16 2x):
                    # out[2b+j] = min(in[4b+j], in[4b+2+j]); after 3 levels
                    # P3[2b+j] = min over {16b+j+2k}; final stride-2 merge
                    # gives exact 16-col block min/max.
                    def plevel(dst, srt, op, cnt):
                        ob = dst[:, 0:1]
                        o = AP(ob.tensor, ob.offset,
                               [list(ob.ap[0]), [2, cnt], [1, 2]])
                        nc.vector.tensor_tensor(o, pairs4(srt, 0, cnt),
                                                pairs4(srt, 2, cnt), op)

                    def plevel_q(dst, srt, op):
                        for qi in range(4):
                            c = (CH[qi + 1] - CH[qi]) // 4
                            ob = dst[:, 0:1]
                            o = AP(ob.tensor, ob.offset + CH[qi] // 2,
                                   [list(ob.ap[0]), [2, c], [1, 2]])
                            nc.vector.tensor_tensor(
                                o, pairs4(srt, CH[qi], c),
                                pairs4(srt, CH[qi] + 2, c), op)

                    plevel_q(p1x, dsH, alu.max)
                    plevel_q(p1n, dsH, alu.min)
                    plevel(p2x, p1x, alu.max, W // 8)
                    plevel(p3x, p2x, alu.max, W // 16)
                    nc.vector.tensor_tensor(b16x[:], stride2(p3x, 0, NB),
                                            stride2(p3x, 1, NB), alu.max)
                    nc.vector.tensor_scalar_mul(nbx[:], b16x[:], -1.0)
                    plevel(p2n, p1n, alu.min, W // 8)
                    plevel(p3n, p2n, alu.min, W // 16)
                    nc.vector.tensor_tensor(b16n[:], stride2(p3n, 0, NB),
                                            stride2(p3n, 1, NB), alu.min)

                    # block-domain sliding-window doubling (fp16 2x),
                    # n/x chains interleaved to hide the write-ack latency
                    nc.vector.tensor_tensor(dx2[:, 0:NB - 1], nbx[:, 0:NB - 1],
                                            nbx[:, 1:NB], alu.min)
                    nc.vector.tensor_tensor(dn2[:, 0:NB - 1], b16n[:, 0:NB - 1],
                                            b16n[:, 1:NB], alu.min)
                    nc.vector.tensor_tensor(dx4[:, 0:NB - 3], dx2[:, 0:NB - 3],
                                            dx2[:, 2:NB - 1], alu.min)
                    nc.vector.tensor_tensor(dn4[:, 0:NB - 3], dn2[:, 0:NB - 3],
                                            dn2[:, 2:NB - 1], alu.min)
                    nc.vector.tensor_tensor(tab[:, NB:2 * NB - 7],
                                            dx4[:, 0:NB - 7],
                                            dx4[:, 4:NB - 3], alu.min)
                    nc.vector.tensor_tensor(tab[:, 0:NB - 7],
                                            dn4[:, 0:NB - 7],
                                            dn4[:, 4:NB - 3], alu.min)
                # ---- DVE: merged min/max lookups; boxes with the same
                # block width share one reduce (extra outer AP dim), and the
                # gather-dependent prefix diffs are slotted in after the
                # first few so the second gather has time to land ----
                def lookup(t, t2=None):
                    b1, b2 = int(xa1[t]), int(xa2[t])
                    q = (b2 - b1) - KB
                    n = q // KB + 1
                    s1 = q - KB * (n - 1)
                    base = tab[:, 0:1]
                    dims = [list(base.ap[0])]
                    odims = [list(rmm[:, 0:1].ap[0])]
                    if t2 is not None:
                        dims.append([int(xa1[t2]) - b1, 2])
                        odims.append([t2 - t, 2])
                    dims.append([NB, 2])
                    odims.append([T + 1, 2])
                    if s1 != 0:
                        dims.append([s1, 2])
                    dims.append([KB, n])
                    v = AP(base.tensor, base.offset + b1, dims)
                    o = AP(rmm.tensor, rmm[:, 0:1].offset + t, odims)
                    nc.vector.tensor_reduce(o, v, XY if s1 != 0 else X,
                                            alu.min)

                # group boxes: pair equal block-widths (equal n and s1)
                from collections import defaultdict
                groups = defaultdict(list)
                for t in range(T):
                    groups[int(xa2[t] - xa1[t])].append(t)
                jobs = []
                for _, ts_ in groups.items():
                    while len(ts_) >= 2:
                        jobs.append((ts_[0], ts_[1]))
                        ts_ = ts_[2:]
                    if ts_:
                        jobs.append((ts_[0], None))

                for t, t2 in jobs[:6]:
                    lookup(t, t2)
                with nc.allow_low_precision(reason="fp16 min/max tables"):
                    nc.vector.tensor_tensor(rs12[:, 0:T], gx12[:, 0:T],
                                            gx12[:, T:2 * T], alu.subtract)
                    nc.vector.tensor_tensor(rrs12[:, 0:T], rs12[:, 0:T],
                                            rindD[:, 0:T], alu.mult)
                    nc.vector.tensor_tensor(rs12[:, T:2 * T], gx12[:, 2 * T:3 * T],
                                            gx12[:, 3 * T:4 * T], alu.subtract)
                    nc.vector.tensor_tensor(rrs12[:, T:2 * T], rs12[:, T:2 * T],
                                            rindD[:, T:2 * T], alu.mult)
                nc.gpsimd.partition_all_reduce(
                    statT[:, 0:2 * T], rrs12[:], 128, RO.add)
                for t, t2 in jobs[6:]:
                    lookup(t, t2)
                # global min/max cands from strided D16 windows
                gb = tab[:, 0:1]
                gpair = list(gb.ap[0])
                gv = AP(gb.tensor, gb.offset, [gpair, [NB, 2], [KB, NB // KB]])
                ob = rmm[:, 0:1]
                opair = list(ob.ap[0])
                og = AP(ob.tensor, ob.offset + T, [opair, [T + 1, 2]])
                nc.vector.tensor_reduce(og, gv, X, alu.min)
                # combine with row mask, negated: stkv = rneg - rmm
                nc.vector.tensor_tensor(stkv[:], rnegS, rmm[:], alu.subtract)
                nc.gpsimd.partition_all_reduce(
                    statT[:, 2 * T:NSTAT], stkv[:], 128, RO.max)

                # ---- exchanges ----
                if single_core or mock_cc:
                    red = statT   # 1-core: gather+reduce is the identity
                else:
                    nc.sync.dma_start(out=cstatS[0:1, :],
                                      in_=statT[0:1, 0:2 * T])
                    nc.gpsimd.collective_compute(
                        "AllGather", alu.bypass,
                        replica_groups=[list(range(NCORES))],
                        ins=[cstatS[:]], outs=[cgathS[:]])
                    nc.sync.dma_start(out=gath[:, 0:2 * T], in_=cgathS[:])
                    nc.scalar.dma_start(out=cstatM[0:1, :],
                                        in_=statT[0:1, 2 * T:NSTAT])
                    nc.gpsimd.collective_compute(
                        "AllGather", alu.bypass,
                        replica_groups=[list(range(NCORES))],
                        ins=[cstatM[:]], outs=[cgathM[:]])
                    nc.scalar.dma_start(out=gath[:, 2 * T:NSTAT],
                                        in_=cgathM[:])
                    nc.gpsimd.partition_all_reduce(
                        redT[:, 0:2 * T], gath[:, 0:2 * T], NCORES, RO.add)
                    nc.gpsimd.partition_all_reduce(
                        redT[:, 2 * T:NSTAT], gath[:, 2 * T:NSTAT],
                        NCORES, RO.max)
                    red = redT

                # ---- final math (row form, partition 0) ----
                # red cols: [2T, 2T+33) = [-bmin | -gmin],
                # [2T+33, 2T+66) = [bmax | gmax]; rng = (-bmin) + bmax
                nc.vector.tensor_tensor(meanR[:], red[0:1, 0:T], cntinvR,
                                        alu.mult)
                nc.vector.tensor_tensor(mBR[:], meanR[:], red[0:1, 0:T],
                                        alu.mult)
                nc.vector.tensor_tensor(a2R[:], red[0:1, T:2 * T], mBR[:],
                                        alu.subtract)
                nc.vector.tensor_tensor(varR[:], a2R[:], cm1invR, alu.mult)
                nc.scalar.sqrt(stdR[:], varR[:])
                nc.tensor.matmul(mcolP[:], meanR[:], oneOne,
                                 start=True, stop=True)
                nc.tensor.matmul(mrB[:], onesRow, meanR[:],
                                 start=True, stop=True)
                # qmd = m_i - m_j, ready before the min/max stats arrive
                nc.vector.tensor_scalar(qm[:], mrB[:], mcolP[:], -1.0,
                                        alu.subtract, alu.mult)
                nc.vector.tensor_tensor(rngR[:], red[0:1, 2 * T:2 * T + T + 1],
                                        red[0:1, 2 * T + T + 1:NSTAT], alu.add)
                nc.vector.reciprocal(rinvR[:], rngR[:])
                # a = 1/(gmax - gmin) broadcast to a [T,1] column via PE
                nc.tensor.matmul(aCol[:], onesRow, rinvR[0:1, T:T + 1],
                                 start=True, stop=True)
                nc.vector.scalar_tensor_tensor(srvR[:], stdR[:], 1.0,
                                               rinvR[0:1, 0:T], alu.mult,
                                               alu.mult,
                                               accum_out=out3[0:1, 1:2])
                nc.vector.scalar_tensor_tensor(t2m[:], qm[:], aCol[:],
                                               gmatC, alu.mult, alu.add)
                nc.vector.scalar_tensor_tensor(t3m[:], t2m[:], 0.0, t2m[:],
                                               alu.max, alu.bypass,
                                               accum_out=raccv[:])
                nc.tensor.matmul(accP[:], raccv[:], ones32c,
                                 start=True, stop=True)
                nc.vector.tensor_copy(out3[0:1, 0:1], accP[0:1, 0:1])
                nc.vector.tensor_tensor(out3[0:1, 2:3], out3[0:1, 0:1],
                                        out3[0:1, 1:2], alu.add)
                nc.sync.dma_start(out=out[:], in_=out3[0:1, 0:3])

    nc.compile()
    return nc


def kernel(d_pred, bboxes, _trace=False):
    from concourse.bass_utils import run_bass_kernel_spmd

    d_pred = np.asarray(d_pred, dtype=np.float32)
    bboxes = np.asarray(bboxes, dtype=np.int32)
    depth = d_pred[0, 0]
    x1, y1, x2, y2 = (bboxes[:, i].astype(np.int64) for i in range(4))

    cnt = ((x2 - x1) * (y2 - y1)).astype(np.float64)
    cntinv = (1.0 / cnt).astype(np.float32)
    cm1inv = (1.0 / (cnt - 1.0)).astype(np.float32)

    ii = np.arange(T)[:, None]
    jj = np.arange(T)[None, :]
    gmat = np.where(jj > ii, (jj - ii) / float(T), -BIG).astype(np.float32)

    cst = np.zeros((128, CST_W), np.float32)
    cst[0:T, 128:160] = gmat
    cst[0, 160:160 + T] = cntinv
    cst[0, 192:192 + T] = cm1inv
    cst[0, 224:224 + T] = 1.0
    cst[:, 226] = 1.0
    # ap_gather indices (shared by both gathers; views are
    # [0 0 | ps] and [0 0 | ps2] with identical relative layout):
    # [hi x 32 | lo x 32]; x1==0 points at the leading zero cols
    PSOFF = 2
    idx = np.empty(2 * T, np.int16)
    idx[0:T] = PSOFF + x2 - 1
    idx[T:2 * T] = np.where(x1 > 0, PSOFF + x1 - 1, 0)
    wrapped = idx.reshape(4, 16).T                      # [16, 4] int16
    cst[:, 256:258] = np.tile(wrapped, (8, 1)).view(np.float32)

    rows = np.arange(H)
    rind_full = ((rows[:, None] >= y1[None, :])
                 & (rows[:, None] < y2[None, :])).astype(np.float32)

    in_maps = []
    for c in range(NCORES):
        ri = rind_full[c * R:(c + 1) * R]          # [R, T]
        rneg = np.zeros((R, NMM), np.float32)
        rneg[:, 0:T] = np.where(ri > 0, 0.0, -BIG)
        rneg[:, T + 1:2 * T + 1] = np.where(ri > 0, 0.0, -BIG)
        din = np.empty((R, DIN_W), np.float32)
        din[:, 0:W] = depth[c * R:(c + 1) * R]
        din[:, W:W + NMM] = rneg
        din[:, W + NMM:W + NMM + T] = ri
        din[:, W + NMM + T:W + NMM + 2 * T] = ri
        in_maps.append({"din": din, "cst": cst})

    nc = _build_program(bboxes)
    res = run_bass_kernel_spmd(nc, in_maps, list(range(NCORES)),
                               trace=_trace)
    o = res.results[0]["out"].astype(np.float32)
    outs = (o[0:1].copy(), o[1:2].copy(), o[2:3].copy())
    if _trace:
        return outs, res
    return outs


# revision 23
# speedup vs baseline: 1.0532x; 1.0532x over previous
"""Trainium2 Bass kernel for the box-ranking depth loss.

Math restructuring (vs the reference):
  - Global min-max normalization is affine; per-box stats of normalized
    depth are recovered from raw-depth stats (the affine constants cancel
    in the loss terms), so each core only needs raw per-box
    {sum, sumsq, min, max} plus the global {min, max}.
  - Box sums/sumsq (exact): per-row f32 prefix sums -> per-box prefix
    differences at the static column edges; the per-box prefix columns
    are fetched with gpsimd ap_gather (one instruction for all 32 boxes)
    -> one subtract + row mask -> cross-row reduction with gpsimd
    partition_all_reduce (no PE transpose round-trip).
  - Box min/max (approximate): column ranges expanded to 16-col block
    boundaries (<= 15 extra cols per side; only perturbs the bmax-bmin
    denominator, ~1e-3 rel on loss_std vs the 2e-2 gate -- dominated by
    the fp16 table rounding, not the expansion).  Block min/max built as
    a j-interleaved packed-pair fp16 pyramid (every level runs at DVE 2x)
    + one stride-2 merge; the max side is negated once at block level so
    block-domain sliding-window doubling and the per-box lookups use MIN
    ops for both sides; ONE strided reduce per box (boxes with equal
    block width share one reduce via an extra AP dim) yields
    (mincand, -maxcand).  All later combines are MAX of negated values.
  - Cross-partition and cross-core combines use partition_all_reduce /
    partition_broadcast; final scalar math is in row form on partition 0.

Sharding: rows (H) split 8 ways -> each core holds a [128, 2048] slab.
Two AllGathers (sums early, min/max late).  Every core redundantly
computes the final 3-float loss vector.
"""

import numpy as np

H, W, T, NCORES = 1024, 2048, 32, 8
R = H // NCORES          # 128 rows per core
BIG = 1e30
RATIO = 1.0
NB = W // 16             # 128 column blocks of 16
KB2 = 16                 # lookup window = 16 blocks = 256 cols
NE = 34                  # lookup entries: 32 boxes + global + pad
NSL = 8                  # gather slots per entry
NMM = 2 * NE             # 68 min/max stat columns
NSTAT = 2 * T + NMM      # 132 total stat columns
DIN_W = W + NMM + 2 * T  # slab | rneg(68) | rinddup(64)
CST_W = 288
PSOFF = 2                # zero cols at the head of ps12 (x1==0 gathers)


def _win_view(tab_ap, b1, b2, k, ap_ctor, pair_stride):
    """AP over block-domain sliding-window tables: windows of k blocks
    covering [b1, b2) (two step-k phases when k does not divide), with an
    outer [pair_stride, 2] dim pairing the min table with the negated max
    table so one MIN reduce serves both sides."""
    q = (b2 - b1) - k
    n = q // k + 1
    s1 = q - k * (n - 1)
    base = tab_ap[:, 0:1]
    ppair = list(base.ap[0])
    dims = [ppair, [pair_stride, 2]]
    if s1 != 0:
        dims.append([s1, 2])
    dims.append([k, n])
    return ap_ctor(base.tensor, base.offset + b1, dims), (s1 != 0)


def _build_program(bboxes, single_core=False, reps=1, mock_cc=False):
    import concourse.bacc as bacc
    import concourse.mybir as mybir
    import concourse.tile as tile
    from concourse.ap import AP
    from concourse.alu_op_type import AluOpType as alu
    from concourse import bass_isa

    f32 = mybir.dt.float32
    f16 = mybir.dt.float16
    X = mybir.AxisListType.X
    XY = mybir.AxisListType.XY
    AF = mybir.ActivationFunctionType
    RO = bass_isa.ReduceOp

    x1s, x2s = bboxes[:, 0], bboxes[:, 2]
    xa1 = (x1s // 16).astype(int)           # block-aligned box edges
    xa2 = ((x2s + 15) // 16).astype(int)
    GVIEW = int(max(x2s)) + PSOFF           # gather view covers all idxs

    nc = bacc.Bacc("TRN2", target_bir_lowering=False, debug=False,
                   num_devices=1 if single_core else NCORES)

    din = nc.dram_tensor("din", [R, DIN_W], f32, kind="ExternalInput").ap()
    cst = nc.dram_tensor("cst", [128, CST_W], f32, kind="ExternalInput").ap()
    out = nc.dram_tensor("out", [3], f32, kind="ExternalOutput").ap()

    def sb(name, shape, dt=f32):
        return nc.alloc_sbuf_tensor(name, shape, dt).ap()

    ds = sb("ds", [R, DIN_W])            # slab + masks
    cstS = sb("cstS", [128, CST_W])
    ds2 = sb("ds2", [R, W])
    ps12 = sb("ps12", [R, 2 * PSOFF + 2 * W])  # [0 0 |ps| 0 0 |ps2]
    dsH = sb("dsH", [R, W], f16)
    p1n = sb("p1n", [R, W // 2], f16)
    p1x = sb("p1x", [R, W // 2], f16)
    p2n = sb("p2n", [R, W // 4], f16)
    p2x = sb("p2x", [R, W // 4], f16)
    p3n = sb("p3n", [R, W // 8], f16)
    p3x = sb("p3x", [R, W // 8], f16)
    b16n = sb("b16n", [R, NB], f16)
    b16x = sb("b16x", [R, NB], f16)
    nbx = sb("nbx", [R, NB], f16)        # negated max blocks
    dn2 = sb("dn2", [R, NB], f16)
    dn4 = sb("dn4", [R, NB], f16)
    dn8 = sb("dn8", [R, NB], f16)
    dx2 = sb("dx2", [R, NB], f16)
    dx4 = sb("dx4", [R, NB], f16)
    dx8 = sb("dx8", [R, NB], f16)
    tabN = sb("tabN", [R, NB])           # f32 D16 min windows + BIG pad
    tabX = sb("tabX", [R, NB])           # f32 D16 neg-max windows + pad
    gxN = sb("gxN", [R, NE * NSL])
    gxX = sb("gxX", [R, NE * NSL])
    gx12 = sb("gx12", [R, 4 * T])
    rs12 = sb("rs12", [R, 2 * T])
    rrs12 = sb("rrs12", [R, 2 * T])
    rmm = sb("rmm", [R, NMM])            # [mincands+gn+pad | -maxcands+gx+pad]
    stkv = sb("stkv", [R, NMM])
    statT = sb("statT", [128, NSTAT])    # PAR outputs: [sums|sumsq|minmax]
    gath = sb("gath", [NCORES, NSTAT])
    redT = sb("redT", [NCORES, NSTAT])
    meanR = sb("meanR", [1, T])
    a2R = sb("a2R", [1, T])
    mBR = sb("mBR", [1, T])
    varR = sb("varR", [1, T])
    stdR = sb("stdR", [1, T])
    rngR = sb("rngR", [1, NE])
    rinvR = sb("rinvR", [1, NE])
    srvR = sb("srvR", [1, T])
    qm = sb("qm", [T, T])
    t2m = sb("t2m", [T, T])
    t3m = sb("t3m", [T, T])
    raccv = sb("raccv", [T, 1])
    dummy = sb("dmy0", [1, 8])
    out3 = sb("out3", [1, 4])

    # const views
    gmatC = cstS[0:T, 128:160]
    cntinvR = cstS[0:1, 160:160 + T]
    cm1invR = cstS[0:1, 192:192 + T]
    onesRow = cstS[0:1, 224:224 + T]
    oneOne = cstS[0:1, 224:225]
    ones32c = cstS[0:T, 226:227]
    idxC = cstS[:, 256:258]
    idxN = cstS[:, 258:267]
    idxX = cstS[:, 267:276]

    rnegS = ds[:, W:W + NMM]
    rindD = ds[:, W + NMM:W + NMM + 2 * T]

    Q = W // 4
    CH = [0, 512, 1024, 1536, 2048]  # DMA/compute chunk bounds

    def pairs4(src, off, cnt):
        # packed-pair view {4b+off, 4b+1+off : b < cnt} -> [R, cnt, 2], 2x
        base = src[:, 0:1]
        pp = list(base.ap[0])
        return AP(base.tensor, base.offset + off, [pp, [4, cnt], [1, 2]])

    def stride2(src, off, cnt):
        base = src[:, 0:1]
        pp = list(base.ap[0])
        return AP(base.tensor, base.offset + off, [pp, [2, cnt]])

    with tile.TileContext(nc) as tc:
        with tc.tile_pool(name="psum", bufs=1, space="PSUM") as pp, \
                tc.tile_pool(name="dram", bufs=1, space="DRAM") as dram:
            mrB = pp.tile([T, T], f32, name="mrB")
            mcolP = pp.tile([T, 1], f32, name="mcolP")
            aCol = pp.tile([T, 1], f32, name="aCol")
            accP = pp.tile([1, 1], f32, name="accP")

            cstatS = dram.tile([1, 2 * T], f32, name="cstatS")
            cgathS = dram.tile([NCORES, 2 * T], f32, name="cgathS")
            cstatM = dram.tile([1, NMM], f32, name="cstatM")
            cgathM = dram.tile([NCORES, NMM], f32, name="cgathM")

            for _rep in range(reps):
                # ---- ACT function-table preloads (overlap the input DMA) ----
                nc.vector.memset(dummy[0:1, 0:1], 0.0)
                nc.scalar.activation(dummy[0:1, 1:2], dummy[0:1, 0:1], AF.Square)
                nc.scalar.activation(dummy[0:1, 2:3], dummy[0:1, 0:1], AF.Sqrt)
                nc.scalar.activation(dummy[0:1, 3:4], dummy[0:1, 0:1], AF.Relu)

                # ---- loads (quarters, alternating the two HWDGE queues) ----
                for qi in range(4):
                    eng = nc.sync if qi % 2 == 0 else nc.scalar
                    eng.dma_start(out=ds[:, CH[qi]:CH[qi + 1]],
                                  in_=din[:, CH[qi]:CH[qi + 1]])
                nc.sync.dma_start(out=ds[:, W:DIN_W], in_=din[:, W:DIN_W])
                nc.scalar.dma_start(out=cstS[:], in_=cst[:])

                # ---- ACT: per-quarter squares and fp16 copies ----
                for qi in range(4):
                    a, b = CH[qi], CH[qi + 1]
                    nc.scalar.square(ds2[:, a:b], ds[:, a:b])
                    nc.scalar.copy(dsH[:, a:b], ds[:, a:b])

                # ---- DVE: row prefix sums (f32 scans) ----
                nc.gpsimd.memset(ps12[:, 0:PSOFF], 0.0)
                nc.gpsimd.memset(tabN[:, NB - 15:NB], BIG)
                nc.gpsimd.memset(tabX[:, NB - 15:NB], BIG)
                nc.gpsimd.memset(ps12[:, PSOFF + W:2 * PSOFF + W], 0.0)
                for qi in range(4):
                    a, b = PSOFF + CH[qi], PSOFF + CH[qi + 1]
                    nc.vector.tensor_tensor_scan(
                        ps12[:, a:b], ds[:, CH[qi]:CH[qi + 1]],
                        ds[:, CH[qi]:CH[qi + 1]],
                        0.0 if qi == 0 else ps12[:, a - 1:a],
                        alu.add, alu.bypass)
                # Pool: gather the sum prefix cols per box (hi x 32 | lo x 32)
                nc.gpsimd.ap_gather(gx12[:, 0:2 * T],
                                    ps12[:, 0:GVIEW],
                                    idxC.bitcast(mybir.dt.int16),
                                    128, GVIEW, 1, 2 * T)
                off2 = 2 * PSOFF + W
                for qi in range(4):
                    a, b = off2 + CH[qi], off2 + CH[qi + 1]
                    nc.vector.tensor_tensor_scan(
                        ps12[:, a:b], ds2[:, CH[qi]:CH[qi + 1]],
                        ds2[:, CH[qi]:CH[qi + 1]],
                        0.0 if qi == 0 else ps12[:, a - 1:a],
                        alu.add, alu.bypass)
                nc.gpsimd.ap_gather(gx12[:, 2 * T:4 * T],
                                    ps12[:, PSOFF + W:PSOFF + W + GVIEW],
                                    idxC.bitcast(mybir.dt.int16),
                                    128, GVIEW, 1, 2 * T)

                # ---- DVE: block min/max pyramid + sliding windows ----
                with nc.allow_low_precision(reason="fp16 min/max tables"):
                    # j-interleaved packed-pair pyramid (fp16 2x):
                    # out[2b+j] = min(in[4b+j], in[4b+2+j]); after 3 levels
                    # P3[2b+j] = min over {16b+j+2k}; final stride-2 merge
                    # gives exact 16-col block min/max.
                    def plevel(dst, srt, op, cnt):
                        ob = dst[:, 0:1]
                        o = AP(ob.tensor, ob.offset,
                               [list(ob.ap[0]), [2, cnt], [1, 2]])
                        nc.vector.tensor_tensor(o, pairs4(srt, 0, cnt),
                                                pairs4(srt, 2, cnt), op)

                    def plevel_q(dst, srt, op):
                        for qi in range(4):
                            c = (CH[qi + 1] - CH[qi]) // 4
                            ob = dst[:, 0:1]
                            o = AP(ob.tensor, ob.offset + CH[qi] // 2,
                                   [list(ob.ap[0]), [2, c], [1, 2]])
                            nc.vector.tensor_tensor(
                                o, pairs4(srt, CH[qi], c),
                                pairs4(srt, CH[qi] + 2, c), op)

                    plevel_q(p1x, dsH, alu.max)
                    plevel_q(p1n, dsH, alu.min)
                    plevel(p2x, p1x, alu.max, W // 8)
                    plevel(p3x, p2x, alu.max, W // 16)
                    nc.vector.tensor_tensor(b16x[:], stride2(p3x, 0, NB),
                                            stride2(p3x, 1, NB), alu.max)
                    nc.vector.tensor_scalar_mul(nbx[:], b16x[:], -1.0)
                    plevel(p2n, p1n, alu.min, W // 8)
                    plevel(p3n, p2n, alu.min, W // 16)
                    nc.vector.tensor_tensor(b16n[:], stride2(p3n, 0, NB),
                                            stride2(p3n, 1, NB), alu.min)

                    # block-domain sliding-window doubling (fp16 2x),
                    # n/x chains interleaved to hide the write-ack latency
                    nc.vector.tensor_tensor(dx2[:, 0:NB - 1], nbx[:, 0:NB - 1],
                                            nbx[:, 1:NB], alu.min)
                    nc.vector.tensor_tensor(dn2[:, 0:NB - 1], b16n[:, 0:NB - 1],
                                            b16n[:, 1:NB], alu.min)
                    nc.vector.tensor_tensor(dx4[:, 0:NB - 3], dx2[:, 0:NB - 3],
                                            dx2[:, 2:NB - 1], alu.min)
                    nc.vector.tensor_tensor(dn4[:, 0:NB - 3], dn2[:, 0:NB - 3],
                                            dn2[:, 2:NB - 1], alu.min)
                    nc.vector.tensor_tensor(dn8[:, 0:NB - 7],
                                            dn4[:, 0:NB - 7],
                                            dn4[:, 4:NB - 3], alu.min)
                    nc.vector.tensor_tensor(dx8[:, 0:NB - 7],
                                            dx4[:, 0:NB - 7],
                                            dx4[:, 4:NB - 3], alu.min)
                    # final 16-block window tables in f32 (ap_gather d=1)
                    nc.vector.tensor_tensor(tabN[:, 0:NB - 15],
                                            dn8[:, 0:NB - 15],
                                            dn8[:, 8:NB - 7], alu.min)
                    nc.vector.tensor_tensor(tabX[:, 0:NB - 15],
                                            dx8[:, 0:NB - 15],
                                            dx8[:, 8:NB - 7], alu.min)
                # ---- min/max lookups: one ap_gather per side fetches all
                # boxes' window values (8 padded slots per entry), then ONE
                # strided reduce per side folds them to per-box cands ----
                nc.gpsimd.ap_gather(gxN[:], tabN[:],
                                    idxN.bitcast(mybir.dt.int16)[:, 0:NE * NSL // 16],
                                    128, NB, 1, NE * NSL)
                nc.gpsimd.ap_gather(gxX[:], tabX[:],
                                    idxX.bitcast(mybir.dt.int16)[:, 0:NE * NSL // 16],
                                    128, NB, 1, NE * NSL)
                with nc.allow_low_precision(reason="fp16 min/max tables"):
                    nc.vector.tensor_tensor(rs12[:, 0:T], gx12[:, 0:T],
                                            gx12[:, T:2 * T], alu.subtract)
                    nc.vector.tensor_tensor(rrs12[:, 0:T], rs12[:, 0:T],
                                            rindD[:, 0:T], alu.mult)
                    nc.vector.tensor_tensor(rs12[:, T:2 * T], gx12[:, 2 * T:3 * T],
                                            gx12[:, 3 * T:4 * T], alu.subtract)
                    nc.vector.tensor_tensor(rrs12[:, T:2 * T], rs12[:, T:2 * T],
                                            rindD[:, T:2 * T], alu.mult)
                nc.gpsimd.partition_all_reduce(
                    statT[:, 0:2 * T], rrs12[:], 128, RO.add)
                gN = gxN[:, 0:1]
                vN = AP(gN.tensor, gN.offset,
                        [list(gN.ap[0]), [NSL, NE], [1, NSL]])
                nc.vector.tensor_reduce(rmm[:, 0:NE], vN, X, alu.min)
                gX = gxX[:, 0:1]
                vX = AP(gX.tensor, gX.offset,
                        [list(gX.ap[0]), [NSL, NE], [1, NSL]])
                nc.vector.tensor_reduce(rmm[:, NE:2 * NE], vX, X, alu.min)
                # combine with row mask, negated: stkv = rneg - rmm
                nc.vector.tensor_tensor(stkv[:], rnegS, rmm[:], alu.subtract)
                nc.gpsimd.partition_all_reduce(
                    statT[:, 2 * T:NSTAT], stkv[:], 128, RO.max)

                # ---- exchanges ----
                if single_core or mock_cc:
                    red = statT   # 1-core: gather+reduce is the identity
                else:
                    nc.sync.dma_start(out=cstatS[0:1, :],
                                      in_=statT[0:1, 0:2 * T])
                    nc.gpsimd.collective_compute(
                        "AllGather", alu.bypass,
                        replica_groups=[list(range(NCORES))],
                        ins=[cstatS[:]], outs=[cgathS[:]])
                    nc.sync.dma_start(out=gath[:, 0:2 * T], in_=cgathS[:])
                    nc.scalar.dma_start(out=cstatM[0:1, :],
                                        in_=statT[0:1, 2 * T:NSTAT])
                    nc.gpsimd.collective_compute(
                        "AllGather", alu.bypass,
                        replica_groups=[list(range(NCORES))],
                        ins=[cstatM[:]], outs=[cgathM[:]])
                    nc.scalar.dma_start(out=gath[:, 2 * T:NSTAT],
                                        in_=cgathM[:])
                    nc.gpsimd.partition_all_reduce(
                        redT[:, 0:2 * T], gath[:, 0:2 * T], NCORES, RO.add)
                    nc.gpsimd.partition_all_reduce(
                        redT[:, 2 * T:NSTAT], gath[:, 2 * T:NSTAT],
                        NCORES, RO.max)
                    red = redT

                # ---- final math (row form, partition 0) ----
                # red cols: [2T, 2T+33) = [-bmin | -gmin],
                # [2T+33, 2T+66) = [bmax | gmax]; rng = (-bmin) + bmax
                nc.vector.tensor_tensor(meanR[:], red[0:1, 0:T], cntinvR,
                                        alu.mult)
                nc.vector.tensor_tensor(mBR[:], meanR[:], red[0:1, 0:T],
                                        alu.mult)
                nc.vector.tensor_tensor(a2R[:], red[0:1, T:2 * T], mBR[:],
                                        alu.subtract)
                nc.vector.tensor_tensor(varR[:], a2R[:], cm1invR, alu.mult)
                nc.scalar.sqrt(stdR[:], varR[:])
                nc.tensor.matmul(mcolP[:], meanR[:], oneOne,
                                 start=True, stop=True)
                nc.tensor.matmul(mrB[:], onesRow, meanR[:],
                                 start=True, stop=True)
                # qmd = m_i - m_j, ready before the min/max stats arrive
                nc.vector.tensor_scalar(qm[:], mrB[:], mcolP[:], -1.0,
                                        alu.subtract, alu.mult)
                nc.vector.tensor_tensor(rngR[:], red[0:1, 2 * T:2 * T + NE],
                                        red[0:1, 2 * T + NE:NSTAT], alu.add)
                nc.vector.reciprocal(rinvR[:], rngR[:])
                # a = 1/(gmax - gmin) broadcast to a [T,1] column via PE
                nc.tensor.matmul(aCol[:], onesRow, rinvR[0:1, T:T + 1],
                                 start=True, stop=True)
                nc.vector.scalar_tensor_tensor(srvR[:], stdR[:], 1.0,
                                               rinvR[0:1, 0:T], alu.mult,
                                               alu.mult,
                                               accum_out=out3[0:1, 1:2])
                nc.vector.scalar_tensor_tensor(t2m[:], qm[:], aCol[:],
                                               gmatC, alu.mult, alu.add)
                nc.vector.scalar_tensor_tensor(t3m[:], t2m[:], 0.0, t2m[:],
                                               alu.max, alu.bypass,
                                               accum_out=raccv[:])
                nc.tensor.matmul(accP[:], raccv[:], ones32c,
                                 start=True, stop=True)
                nc.vector.tensor_copy(out3[0:1, 0:1], accP[0:1, 0:1])
                nc.vector.tensor_tensor(out3[0:1, 2:3], out3[0:1, 0:1],
                                        out3[0:1, 1:2], alu.add)
                nc.sync.dma_start(out=out[:], in_=out3[0:1, 0:3])

    nc.compile()
    return nc


def kernel(d_pred, bboxes, _trace=False):
    from concourse.bass_utils import run_bass_kernel_spmd
    NE = 34

    d_pred = np.asarray(d_pred, dtype=np.float32)
    bboxes = np.asarray(bboxes, dtype=np.int32)
    depth = d_pred[0, 0]
    x1, y1, x2, y2 = (bboxes[:, i].astype(np.int64) for i in range(4))

    cnt = ((x2 - x1) * (y2 - y1)).astype(np.float64)
    cntinv = (1.0 / cnt).astype(np.float32)
    cm1inv = (1.0 / (cnt - 1.0)).astype(np.float32)

    ii = np.arange(T)[:, None]
    jj = np.arange(T)[None, :]
    gmat = np.where(jj > ii, (jj - ii) / float(T), -BIG).astype(np.float32)

    cst = np.zeros((128, CST_W), np.float32)
    cst[0:T, 128:160] = gmat
    cst[0, 160:160 + T] = cntinv
    cst[0, 192:192 + T] = cm1inv
    cst[0, 224:224 + T] = 1.0
    cst[:, 226] = 1.0
    # ap_gather indices (shared by both gathers; views are
    # [0 0 | ps] and [0 0 | ps2] with identical relative layout):
    # [hi x 32 | lo x 32]; x1==0 points at the leading zero cols
    PSOFF = 2
    idx = np.empty(2 * T, np.int16)
    idx[0:T] = PSOFF + x2 - 1
    idx[T:2 * T] = np.where(x1 > 0, PSOFF + x1 - 1, 0)
    wrapped = idx.reshape(4, 16).T                      # [16, 4] int16
    cst[:, 256:258] = np.tile(wrapped, (8, 1)).view(np.float32)

    # lookup gather indices: per entry 8 slots of 16-block window starts
    # (two phases), padded with NB-1 (a BIG column); entry 32 = global
    # strided windows; entry 33 = all-pad dummy
    NB_, KB2_, NE_, NSL_ = W // 16, 16, 34, 8
    xa1 = (x1 // 16).astype(int)
    xa2 = ((x2 + 15) // 16).astype(int)
    lidx = np.full((NE_, NSL_), NB_ - 1, np.int16)
    for t in range(T):
        b1, b2 = int(xa1[t]), int(xa2[t])
        q = (b2 - b1) - KB2_
        n = q // KB2_ + 1
        s1 = q - KB2_ * (n - 1)
        offs = [b1 + KB2_ * i for i in range(n)]
        if s1 != 0:
            offs += [b1 + s1 + KB2_ * i for i in range(n)]
        lidx[t, 0:len(offs)] = offs
    lidx[32, :] = np.arange(8) * KB2_
    lflat = lidx.reshape(-1)                       # 272 idxs
    lwrap = np.tile(lflat.reshape(-1, 16).T, (8, 1))   # [128, 17] int16
    lw = np.zeros((128, 18), np.int16)
    lw[:, 0:17] = lwrap
    cst[:, 258:267] = lw.view(np.float32)
    cst[:, 267:276] = lw.view(np.float32)

    rows = np.arange(H)
    rind_full = ((rows[:, None] >= y1[None, :])
                 & (rows[:, None] < y2[None, :])).astype(np.float32)

    in_maps = []
    for c in range(NCORES):
        ri = rind_full[c * R:(c + 1) * R]          # [R, T]
        rneg = np.full((R, NMM), -BIG, np.float32)
        rneg[:, 0:T] = np.where(ri > 0, 0.0, -BIG)
        rneg[:, 32] = 0.0
        rneg[:, NE + 0:NE + T] = np.where(ri > 0, 0.0, -BIG)
        rneg[:, NE + 32] = 0.0
        din = np.empty((R, DIN_W), np.float32)
        din[:, 0:W] = depth[c * R:(c + 1) * R]
        din[:, W:W + NMM] = rneg
        din[:, W + NMM:W + NMM + T] = ri
        din[:, W + NMM + T:W + NMM + 2 * T] = ri
        in_maps.append({"din": din, "cst": cst})

    nc = _build_program(bboxes)
    res = run_bass_kernel_spmd(nc, in_maps, list(range(NCORES)),
                               trace=_trace)
    o = res.results[0]["out"].astype(np.float32)
    outs = (o[0:1].copy(), o[1:2].copy(), o[2:3].copy())
    if _trace:
        return outs, res
    return outs


# revision 24
# speedup vs baseline: 1.0605x; 1.0069x over previous
"""Trainium2 Bass kernel for the box-ranking depth loss.

Math restructuring (vs the reference):
  - Global min-max normalization is affine; per-box stats of normalized
    depth are recovered from raw-depth stats (the affine constants cancel
    in the loss terms), so each core only needs raw per-box
    {sum, sumsq, min, max} plus the global {min, max}.
  - Box sums/sumsq (exact): per-row f32 prefix sums -> per-box prefix
    differences at the static column edges; the per-box prefix columns
    are fetched with gpsimd ap_gather (one instruction for all 32 boxes)
    -> one subtract + row mask -> cross-row reduction with gpsimd
    partition_all_reduce (no PE transpose round-trip).
  - Box min/max (approximate): column ranges expanded to 16-col block
    boundaries (<= 15 extra cols per side; only perturbs the bmax-bmin
    denominator, ~1e-3 rel on loss_std vs the 2e-2 gate -- dominated by
    the fp16 table rounding, not the expansion).  Block min/max built as
    a j-interleaved packed-pair fp16 pyramid (every level runs at DVE 2x)
    + one stride-2 merge; the max side is negated once at block level so
    block-domain sliding-window doubling and the per-box lookups use MIN
    ops for both sides; ONE strided reduce per box (boxes with equal
    block width share one reduce via an extra AP dim) yields
    (mincand, -maxcand).  All later combines are MAX of negated values.
  - Cross-partition and cross-core combines use partition_all_reduce /
    partition_broadcast; final scalar math is in row form on partition 0.

Sharding: rows (H) split 8 ways -> each core holds a [128, 2048] slab.
Two AllGathers (sums early, min/max late).  Every core redundantly
computes the final 3-float loss vector.
"""

import numpy as np

H, W, T, NCORES = 1024, 2048, 32, 8
R = H // NCORES          # 128 rows per core
BIG = 1e30
RATIO = 1.0
NB = W // 16             # 128 column blocks of 16
KB2 = 16                 # lookup window = 16 blocks = 256 cols
NE = 34                  # lookup entries: 32 boxes + global + pad
NSL = 8                  # gather slots per entry
NMM = 2 * NE             # 68 min/max stat columns
NSTAT = 2 * T + NMM      # 132 total stat columns
DIN_W = W + NMM + 2 * T  # slab | rneg(68) | rinddup(64)
CST_W = 288
PSOFF = 2                # zero cols at the head of ps12 (x1==0 gathers)


def _win_view(tab_ap, b1, b2, k, ap_ctor, pair_stride):
    """AP over block-domain sliding-window tables: windows of k blocks
    covering [b1, b2) (two step-k phases when k does not divide), with an
    outer [pair_stride, 2] dim pairing the min table with the negated max
    table so one MIN reduce serves both sides."""
    q = (b2 - b1) - k
    n = q // k + 1
    s1 = q - k * (n - 1)
    base = tab_ap[:, 0:1]
    ppair = list(base.ap[0])
    dims = [ppair, [pair_stride, 2]]
    if s1 != 0:
        dims.append([s1, 2])
    dims.append([k, n])
    return ap_ctor(base.tensor, base.offset + b1, dims), (s1 != 0)


def _build_program(bboxes, single_core=False, reps=1, mock_cc=False):
    import concourse.bacc as bacc
    import concourse.mybir as mybir
    import concourse.tile as tile
    from concourse.ap import AP
    from concourse.alu_op_type import AluOpType as alu
    from concourse import bass_isa

    f32 = mybir.dt.float32
    f16 = mybir.dt.float16
    X = mybir.AxisListType.X
    XY = mybir.AxisListType.XY
    AF = mybir.ActivationFunctionType
    RO = bass_isa.ReduceOp

    x1s, x2s = bboxes[:, 0], bboxes[:, 2]
    xa1 = (x1s // 16).astype(int)           # block-aligned box edges
    xa2 = ((x2s + 15) // 16).astype(int)
    GVIEW = int(max(x2s)) + PSOFF           # gather view covers all idxs

    nc = bacc.Bacc("TRN2", target_bir_lowering=False, debug=False,
                   num_devices=1 if single_core else NCORES)

    din = nc.dram_tensor("din", [R, DIN_W], f32, kind="ExternalInput").ap()
    cst = nc.dram_tensor("cst", [128, CST_W], f32, kind="ExternalInput").ap()
    out = nc.dram_tensor("out", [3], f32, kind="ExternalOutput").ap()

    def sb(name, shape, dt=f32):
        return nc.alloc_sbuf_tensor(name, shape, dt).ap()

    ds = sb("ds", [R, DIN_W])            # slab + masks
    cstS = sb("cstS", [128, CST_W])
    ds2 = sb("ds2", [R, W])
    ps12 = sb("ps12", [R, 2 * PSOFF + 2 * W])  # [0 0 |ps| 0 0 |ps2]
    dsH = sb("dsH", [R, W], f16)
    p1n = sb("p1n", [R, W // 2], f16)
    p1x = sb("p1x", [R, W // 2], f16)
    p2n = sb("p2n", [R, W // 4], f16)
    p2x = sb("p2x", [R, W // 4], f16)
    p3n = sb("p3n", [R, W // 8], f16)
    p3x = sb("p3x", [R, W // 8], f16)
    b16n = sb("b16n", [R, NB], f16)
    b16x = sb("b16x", [R, NB], f16)
    nbx = sb("nbx", [R, NB], f16)        # negated max blocks
    dn2 = sb("dn2", [R, NB], f16)
    dn4 = sb("dn4", [R, NB], f16)
    dn8 = sb("dn8", [R, NB], f16)
    dx2 = sb("dx2", [R, NB], f16)
    dx4 = sb("dx4", [R, NB], f16)
    dx8 = sb("dx8", [R, NB], f16)
    tabN = sb("tabN", [R, NB])           # f32 D16 min windows + BIG pad
    tabX = sb("tabX", [R, NB])           # f32 D16 neg-max windows + pad
    gxN = sb("gxN", [R, NE * NSL])
    gxX = sb("gxX", [R, NE * NSL])
    gx12 = sb("gx12", [R, 4 * T])
    rs12 = sb("rs12", [R, 2 * T])
    rrs12 = sb("rrs12", [R, 2 * T])
    rmm = sb("rmm", [R, NMM])            # [mincands+gn+pad | -maxcands+gx+pad]
    stkv = sb("stkv", [R, NMM])
    statT = sb("statT", [128, NSTAT])    # PAR outputs: [sums|sumsq|minmax]
    gath = sb("gath", [NCORES, NSTAT])
    redT = sb("redT", [NCORES, NSTAT])
    meanR = sb("meanR", [1, T])
    a2R = sb("a2R", [1, T])
    mBR = sb("mBR", [1, T])
    varR = sb("varR", [1, T])
    stdR = sb("stdR", [1, T])
    rngR = sb("rngR", [1, NE])
    rinvR = sb("rinvR", [1, NE])
    srvR = sb("srvR", [1, T])
    qm = sb("qm", [T, T])
    t2m = sb("t2m", [T, T])
    t3m = sb("t3m", [T, T])
    raccv = sb("raccv", [T, 1])
    dummy = sb("dmy0", [1, 8])
    out3 = sb("out3", [1, 4])

    # const views
    gmatC = cstS[0:T, 128:160]
    cntinvR = cstS[0:1, 160:160 + T]
    cm1invR = cstS[0:1, 192:192 + T]
    onesRow = cstS[0:1, 224:224 + T]
    oneOne = cstS[0:1, 224:225]
    ones32c = cstS[0:T, 226:227]
    idxC = cstS[:, 256:258]
    idxN = cstS[:, 258:267]
    idxX = cstS[:, 267:276]

    rnegS = ds[:, W:W + NMM]
    rindD = ds[:, W + NMM:W + NMM + 2 * T]

    Q = W // 4
    CH = [0, 512, 1024, 1536, 2048]  # DMA/compute chunk bounds

    def pairs4(src, off, cnt):
        # packed-pair view {4b+off, 4b+1+off : b < cnt} -> [R, cnt, 2], 2x
        base = src[:, 0:1]
        pp = list(base.ap[0])
        return AP(base.tensor, base.offset + off, [pp, [4, cnt], [1, 2]])

    def stride2(src, off, cnt):
        base = src[:, 0:1]
        pp = list(base.ap[0])
        return AP(base.tensor, base.offset + off, [pp, [2, cnt]])

    with tile.TileContext(nc) as tc:
        with tc.tile_pool(name="psum", bufs=1, space="PSUM") as pp, \
                tc.tile_pool(name="dram", bufs=1, space="DRAM") as dram:
            mrB = pp.tile([T, T], f32, name="mrB")
            mcolP = pp.tile([T, 1], f32, name="mcolP")
            aCol = pp.tile([T, 1], f32, name="aCol")
            accP = pp.tile([1, 1], f32, name="accP")

            cstatS = dram.tile([1, 2 * T], f32, name="cstatS")
            cgathS = dram.tile([NCORES, 2 * T], f32, name="cgathS")
            cstatM = dram.tile([1, NMM], f32, name="cstatM")
            cgathM = dram.tile([NCORES, NMM], f32, name="cgathM")

            for _rep in range(reps):
                # ---- ACT function-table preloads (overlap the input DMA) ----
                nc.vector.memset(dummy[0:1, 0:1], 0.0)
                nc.scalar.activation(dummy[0:1, 1:2], dummy[0:1, 0:1], AF.Square)
                nc.scalar.activation(dummy[0:1, 2:3], dummy[0:1, 0:1], AF.Sqrt)
                nc.scalar.activation(dummy[0:1, 3:4], dummy[0:1, 0:1], AF.Relu)

                # ---- loads (quarters, alternating the two HWDGE queues) ----
                for qi in range(4):
                    eng = nc.sync if qi % 2 == 0 else nc.scalar
                    eng.dma_start(out=ds[:, CH[qi]:CH[qi + 1]],
                                  in_=din[:, CH[qi]:CH[qi + 1]])
                nc.sync.dma_start(out=ds[:, W:DIN_W], in_=din[:, W:DIN_W])
                nc.scalar.dma_start(out=cstS[:], in_=cst[:])

                # ---- ACT: per-quarter squares and fp16 copies ----
                for qi in range(4):
                    a, b = CH[qi], CH[qi + 1]
                    nc.scalar.square(ds2[:, a:b], ds[:, a:b])
                    nc.scalar.copy(dsH[:, a:b], ds[:, a:b])

                # ---- DVE: row prefix sums (f32 scans) ----
                nc.gpsimd.memset(ps12[:, 0:PSOFF], 0.0)
                nc.gpsimd.memset(tabN[:, NB - 15:NB], BIG)
                nc.gpsimd.memset(tabX[:, NB - 15:NB], BIG)
                nc.gpsimd.memset(ps12[:, PSOFF + W:2 * PSOFF + W], 0.0)
                for qi in range(4):
                    a, b = PSOFF + CH[qi], PSOFF + CH[qi + 1]
                    nc.vector.tensor_tensor_scan(
                        ps12[:, a:b], ds[:, CH[qi]:CH[qi + 1]],
                        ds[:, CH[qi]:CH[qi + 1]],
                        0.0 if qi == 0 else ps12[:, a - 1:a],
                        alu.add, alu.bypass)
                # Pool: gather the sum prefix cols per box (hi x 32 | lo x 32)
                nc.gpsimd.ap_gather(gx12[:, 0:2 * T],
                                    ps12[:, 0:GVIEW],
                                    idxC.bitcast(mybir.dt.int16),
                                    128, GVIEW, 1, 2 * T)
                off2 = 2 * PSOFF + W
                for qi in range(4):
                    a, b = off2 + CH[qi], off2 + CH[qi + 1]
                    nc.vector.tensor_tensor_scan(
                        ps12[:, a:b], ds2[:, CH[qi]:CH[qi + 1]],
                        ds2[:, CH[qi]:CH[qi + 1]],
                        0.0 if qi == 0 else ps12[:, a - 1:a],
                        alu.add, alu.bypass)
                nc.gpsimd.ap_gather(gx12[:, 2 * T:4 * T],
                                    ps12[:, PSOFF + W:PSOFF + W + GVIEW],
                                    idxC.bitcast(mybir.dt.int16),
                                    128, GVIEW, 1, 2 * T)

                # ---- DVE: block min/max pyramid + sliding windows ----
                with nc.allow_low_precision(reason="fp16 min/max tables"):
                    # j-interleaved packed-pair pyramid (fp16 2x):
                    # out[2b+j] = min(in[4b+j], in[4b+2+j]); after 3 levels
                    # P3[2b+j] = min over {16b+j+2k}; final stride-2 merge
                    # gives exact 16-col block min/max.
                    def plevel(dst, srt, op, cnt):
                        ob = dst[:, 0:1]
                        o = AP(ob.tensor, ob.offset,
                               [list(ob.ap[0]), [2, cnt], [1, 2]])
                        nc.vector.tensor_tensor(o, pairs4(srt, 0, cnt),
                                                pairs4(srt, 2, cnt), op)

                    def plevel_q(dst, srt, op):
                        for qi in range(4):
                            c = (CH[qi + 1] - CH[qi]) // 4
                            ob = dst[:, 0:1]
                            o = AP(ob.tensor, ob.offset + CH[qi] // 2,
                                   [list(ob.ap[0]), [2, c], [1, 2]])
                            nc.vector.tensor_tensor(
                                o, pairs4(srt, CH[qi], c),
                                pairs4(srt, CH[qi] + 2, c), op)

                    plevel_q(p1x, dsH, alu.max)
                    plevel_q(p1n, dsH, alu.min)
                    plevel(p2x, p1x, alu.max, W // 8)
                    plevel(p3x, p2x, alu.max, W // 16)
                    nc.vector.tensor_tensor(b16x[:], stride2(p3x, 0, NB),
                                            stride2(p3x, 1, NB), alu.max)
                    nc.vector.tensor_scalar_mul(nbx[:], b16x[:], -1.0)
                    plevel(p2n, p1n, alu.min, W // 8)
                    plevel(p3n, p2n, alu.min, W // 16)
                    nc.vector.tensor_tensor(b16n[:], stride2(p3n, 0, NB),
                                            stride2(p3n, 1, NB), alu.min)

                    # block-domain sliding-window doubling (fp16 2x),
                    # n/x chains interleaved to hide the write-ack latency
                    nc.vector.tensor_tensor(dn2[:, 0:NB - 1], b16n[:, 0:NB - 1],
                                            b16n[:, 1:NB], alu.min)
                    nc.vector.tensor_tensor(dx2[:, 0:NB - 1], nbx[:, 0:NB - 1],
                                            nbx[:, 1:NB], alu.min)
                    nc.vector.tensor_tensor(dn4[:, 0:NB - 3], dn2[:, 0:NB - 3],
                                            dn2[:, 2:NB - 1], alu.min)
                    nc.vector.tensor_tensor(dn8[:, 0:NB - 7],
                                            dn4[:, 0:NB - 7],
                                            dn4[:, 4:NB - 3], alu.min)
                    nc.vector.tensor_tensor(dx4[:, 0:NB - 3], dx2[:, 0:NB - 3],
                                            dx2[:, 2:NB - 1], alu.min)
                    # final 16-block window tables in f32 (ap_gather d=1)
                    nc.vector.tensor_tensor(tabN[:, 0:NB - 15],
                                            dn8[:, 0:NB - 15],
                                            dn8[:, 8:NB - 7], alu.min)
                    nc.vector.tensor_tensor(dx8[:, 0:NB - 7],
                                            dx4[:, 0:NB - 7],
                                            dx4[:, 4:NB - 3], alu.min)
                    nc.vector.tensor_tensor(tabX[:, 0:NB - 15],
                                            dx8[:, 0:NB - 15],
                                            dx8[:, 8:NB - 7], alu.min)
                # ---- min/max lookups: one ap_gather per side fetches all
                # boxes' window values (8 padded slots per entry), then ONE
                # strided reduce per side folds them to per-box cands ----
                nc.gpsimd.ap_gather(gxN[:], tabN[:],
                                    idxN.bitcast(mybir.dt.int16)[:, 0:NE * NSL // 16],
                                    128, NB, 1, NE * NSL)
                nc.gpsimd.ap_gather(gxX[:], tabX[:],
                                    idxX.bitcast(mybir.dt.int16)[:, 0:NE * NSL // 16],
                                    128, NB, 1, NE * NSL)
                with nc.allow_low_precision(reason="fp16 min/max tables"):
                    nc.vector.tensor_tensor(rs12[:, 0:T], gx12[:, 0:T],
                                            gx12[:, T:2 * T], alu.subtract)
                    nc.vector.tensor_tensor(rrs12[:, 0:T], rs12[:, 0:T],
                                            rindD[:, 0:T], alu.mult)
                    nc.vector.tensor_tensor(rs12[:, T:2 * T], gx12[:, 2 * T:3 * T],
                                            gx12[:, 3 * T:4 * T], alu.subtract)
                    nc.vector.tensor_tensor(rrs12[:, T:2 * T], rs12[:, T:2 * T],
                                            rindD[:, T:2 * T], alu.mult)
                nc.gpsimd.partition_all_reduce(
                    statT[:, 0:2 * T], rrs12[:], 128, RO.add)
                gN = gxN[:, 0:1]
                vN = AP(gN.tensor, gN.offset,
                        [list(gN.ap[0]), [NSL, NE], [1, NSL]])
                nc.vector.tensor_reduce(rmm[:, 0:NE], vN, X, alu.min)
                gX = gxX[:, 0:1]
                vX = AP(gX.tensor, gX.offset,
                        [list(gX.ap[0]), [NSL, NE], [1, NSL]])
                nc.vector.tensor_reduce(rmm[:, NE:2 * NE], vX, X, alu.min)
                # combine with row mask, negated: stkv = rneg - rmm
                nc.vector.tensor_tensor(stkv[:], rnegS, rmm[:], alu.subtract)
                nc.gpsimd.partition_all_reduce(
                    statT[:, 2 * T:NSTAT], stkv[:], 128, RO.max)

                # ---- exchanges ----
                if single_core or mock_cc:
                    red = statT   # 1-core: gather+reduce is the identity
                else:
                    nc.sync.dma_start(out=cstatS[0:1, :],
                                      in_=statT[0:1, 0:2 * T])
                    nc.gpsimd.collective_compute(
                        "AllGather", alu.bypass,
                        replica_groups=[list(range(NCORES))],
                        ins=[cstatS[:]], outs=[cgathS[:]])
                    nc.sync.dma_start(out=gath[:, 0:2 * T], in_=cgathS[:])
                    nc.scalar.dma_start(out=cstatM[0:1, :],
                                        in_=statT[0:1, 2 * T:NSTAT])
                    nc.gpsimd.collective_compute(
                        "AllGather", alu.bypass,
                        replica_groups=[list(range(NCORES))],
                        ins=[cstatM[:]], outs=[cgathM[:]])
                    nc.scalar.dma_start(out=gath[:, 2 * T:NSTAT],
                                        in_=cgathM[:])
                    nc.gpsimd.partition_all_reduce(
                        redT[:, 0:2 * T], gath[:, 0:2 * T], NCORES, RO.add)
                    nc.gpsimd.partition_all_reduce(
                        redT[:, 2 * T:NSTAT], gath[:, 2 * T:NSTAT],
                        NCORES, RO.max)
                    red = redT

                # ---- final math (row form, partition 0) ----
                # red cols: [2T, 2T+33) = [-bmin | -gmin],
                # [2T+33, 2T+66) = [bmax | gmax]; rng = (-bmin) + bmax
                nc.vector.tensor_tensor(rngR[:], red[0:1, 2 * T:2 * T + NE],
                                        red[0:1, 2 * T + NE:NSTAT], alu.add)
                nc.vector.reciprocal(rinvR[:], rngR[:])
                nc.vector.tensor_tensor(meanR[:], red[0:1, 0:T], cntinvR,
                                        alu.mult)
                nc.vector.tensor_tensor(mBR[:], meanR[:], red[0:1, 0:T],
                                        alu.mult)
                nc.vector.tensor_tensor(a2R[:], red[0:1, T:2 * T], mBR[:],
                                        alu.subtract)
                nc.vector.tensor_tensor(varR[:], a2R[:], cm1invR, alu.mult)
                nc.scalar.sqrt(stdR[:], varR[:])
                nc.tensor.matmul(mcolP[:], meanR[:], oneOne,
                                 start=True, stop=True)
                nc.tensor.matmul(mrB[:], onesRow, meanR[:],
                                 start=True, stop=True)
                # qmd = m_i - m_j
                nc.vector.tensor_scalar(qm[:], mrB[:], mcolP[:], -1.0,
                                        alu.subtract, alu.mult)
                # a = 1/(gmax - gmin) broadcast to a [T,1] column via PE
                nc.tensor.matmul(aCol[:], onesRow, rinvR[0:1, T:T + 1],
                                 start=True, stop=True)
                nc.vector.scalar_tensor_tensor(srvR[:], stdR[:], 1.0,
                                               rinvR[0:1, 0:T], alu.mult,
                                               alu.mult,
                                               accum_out=out3[0:1, 1:2])
                nc.vector.scalar_tensor_tensor(t2m[:], qm[:], aCol[:],
                                               gmatC, alu.mult, alu.add)
                nc.vector.scalar_tensor_tensor(t3m[:], t2m[:], 0.0, t2m[:],
                                               alu.max, alu.bypass,
                                               accum_out=raccv[:])
                nc.tensor.matmul(accP[:], raccv[:], ones32c,
                                 start=True, stop=True)
                nc.vector.tensor_copy(out3[0:1, 0:1], accP[0:1, 0:1])
                nc.vector.tensor_tensor(out3[0:1, 2:3], out3[0:1, 0:1],
                                        out3[0:1, 1:2], alu.add)
                nc.sync.dma_start(out=out[:], in_=out3[0:1, 0:3])

    nc.compile()
    return nc


def kernel(d_pred, bboxes, _trace=False):
    from concourse.bass_utils import run_bass_kernel_spmd
    NE = 34

    d_pred = np.asarray(d_pred, dtype=np.float32)
    bboxes = np.asarray(bboxes, dtype=np.int32)
    depth = d_pred[0, 0]
    x1, y1, x2, y2 = (bboxes[:, i].astype(np.int64) for i in range(4))

    cnt = ((x2 - x1) * (y2 - y1)).astype(np.float64)
    cntinv = (1.0 / cnt).astype(np.float32)
    cm1inv = (1.0 / (cnt - 1.0)).astype(np.float32)

    ii = np.arange(T)[:, None]
    jj = np.arange(T)[None, :]
    gmat = np.where(jj > ii, (jj - ii) / float(T), -BIG).astype(np.float32)

    cst = np.zeros((128, CST_W), np.float32)
    cst[0:T, 128:160] = gmat
    cst[0, 160:160 + T] = cntinv
    cst[0, 192:192 + T] = cm1inv
    cst[0, 224:224 + T] = 1.0
    cst[:, 226] = 1.0
    # ap_gather indices (shared by both gathers; views are
    # [0 0 | ps] and [0 0 | ps2] with identical relative layout):
    # [hi x 32 | lo x 32]; x1==0 points at the leading zero cols
    PSOFF = 2
    idx = np.empty(2 * T, np.int16)
    idx[0:T] = PSOFF + x2 - 1
    idx[T:2 * T] = np.where(x1 > 0, PSOFF + x1 - 1, 0)
    wrapped = idx.reshape(4, 16).T                      # [16, 4] int16
    cst[:, 256:258] = np.tile(wrapped, (8, 1)).view(np.float32)

    # lookup gather indices: per entry 8 slots of 16-block window starts
    # (two phases), padded with NB-1 (a BIG column); entry 32 = global
    # strided windows; entry 33 = all-pad dummy
    NB_, KB2_, NE_, NSL_ = W // 16, 16, 34, 8
    xa1 = (x1 // 16).astype(int)
    xa2 = ((x2 + 15) // 16).astype(int)
    lidx = np.full((NE_, NSL_), NB_ - 1, np.int16)
    for t in range(T):
        b1, b2 = int(xa1[t]), int(xa2[t])
        q = (b2 - b1) - KB2_
        n = q // KB2_ + 1
        s1 = q - KB2_ * (n - 1)
        offs = [b1 + KB2_ * i for i in range(n)]
        if s1 != 0:
            offs += [b1 + s1 + KB2_ * i for i in range(n)]
        lidx[t, 0:len(offs)] = offs
    lidx[32, :] = np.arange(8) * KB2_
    lflat = lidx.reshape(-1)                       # 272 idxs
    lwrap = np.tile(lflat.reshape(-1, 16).T, (8, 1))   # [128, 17] int16
    lw = np.zeros((128, 18), np.int16)
    lw[:, 0:17] = lwrap
    cst[:, 258:267] = lw.view(np.float32)
    cst[:, 267:276] = lw.view(np.float32)

    rows = np.arange(H)
    rind_full = ((rows[:, None] >= y1[None, :])
                 & (rows[:, None] < y2[None, :])).astype(np.float32)

    in_maps = []
    for c in range(NCORES):
        ri = rind_full[c * R:(c + 1) * R]          # [R, T]
        rneg = np.full((R, NMM), -BIG, np.float32)
        rneg[:, 0:T] = np.where(ri > 0, 0.0, -BIG)
        rneg[:, 32] = 0.0
        rneg[:, NE + 0:NE + T] = np.where(ri > 0, 0.0, -BIG)
        rneg[:, NE + 32] = 0.0
        din = np.empty((R, DIN_W), np.float32)
        din[:, 0:W] = depth[c * R:(c + 1) * R]
        din[:, W:W + NMM] = rneg
        din[:, W + NMM:W + NMM + T] = ri
        din[:, W + NMM + T:W + NMM + 2 * T] = ri
        in_maps.append({"din": din, "cst": cst})

    nc = _build_program(bboxes)
    res = run_bass_kernel_spmd(nc, in_maps, list(range(NCORES)),
                               trace=_trace)
    o = res.results[0]["out"].astype(np.float32)
    outs = (o[0:1].copy(), o[1:2].copy(), o[2:3].copy())
    if _trace:
        return outs, res
    return outs
